# revision 30
# baseline (speedup 1.0000x reference)
"""DropToken gather kernel for Trainium2 (8 NeuronCores).

Computes out[b, c, :] = inputs[b, idx[c], :] (the reference's one-hot
matmul is just a row gather). Memory-bound.

Key layout trick: all 4 batches share idx, so the host interleaves
batches into x_il[l] = concat(x[0,l], x[1,l], x[2,l], x[3,l]) — one 8KB
fp16 row per token. One SWDGE gather descriptor then fetches the row
for all 4 batches at once: 512 descriptors/core instead of 2048, and
8KB packets keep the 16 SDMA engines at full per-packet efficiency.

Precision: correctness gate is rel_err < 2e-2; fp16 rounding costs
~5e-4 while halving every DMA byte. Host casts f32->f16 and back.

Sharding: core k handles output rows [k*512, (k+1)*512) of the cap dim
for all batches. Slot (p, t) of the [128, T=4] layout = row p*T + t.
"""

import numpy as np

import concourse.bass as bass
import concourse.tile as tile
from concourse import bacc, mybir
from concourse.bass_utils import run_bass_kernel_spmd

B = 4
LENGTH = 8192
EMBED = 1024
CAP = 4096
N_CORES = 8
WIDTH = B * EMBED  # interleaved row width (elements)
ROWS_PER_CORE = CAP // N_CORES  # 512 cap rows per core
T = ROWS_PER_CORE // 128  # 4 gathered rows per partition

DT = mybir.dt.float16
NP_DT = np.float16

_nc_cache = None
STRIP_INIT_BARRIER = True
MODE = "tile"  # "dram" = single-pass HBM->HBM gather (crashes: SWDGE ucode
               # computes partition-style dst addresses; DRAM dest unsupported)
N_SWDGE_QUEUES = 2
IDX_ON_GPSIMD = True

# Each token row (8KB) is gathered as two half-rows (x viewed as
# [2*LENGTH, WIDTH/2], indices 2i and 2i+1): completion granularity
# halves to 0.52MB so the tail store after the last gather is short.
HALF = 2
HALFW = WIDTH // HALF

# Gathers stay full-width (SWDGE offset APs at partition offsets crash
# the runtime). Stores: the 16 SDMA engines are saturated mid-kernel
# (~360 GB/s DMA bus), so what matters is the tail — every store piece is
# partition-split across the sync/scalar HWDGE rings, and the final
# half-column also borrows gpsimd's SWDGE queue as a third feed (its
# descriptor-gen work is long done by then).


def _strip_init_barrier(nc):
    """Remove the Bass-init const memsets and all-engine barrier from the
    entry block. This kernel has no cross-engine deps besides DMA
    semaphores (runtime-zeroed at NEFF load), so engine-boot alignment is
    unnecessary; saves ~3us of startup."""
    blk = nc.m.functions[0].blocks[0]
    blk.instructions = [
        ins
        for ins in blk.instructions
        if not isinstance(
            ins, (mybir.InstMemset, mybir.InstDrain, mybir.InstEventSemaphore)
        )
    ]


def _indirect_gather(eng, out_ap, in_ap, offset_ap, queue_num=0, oob_is_err=True):
    """Indirect gather (one offset per partition) pinned to
    qPoolDynamic{queue_num}, allowing any (incl. DRAM) destination AP.
    Mirrors bass's indirect_dma_start gather-arm lowering."""
    out_l = eng.lower_ap_dma(out_ap, for_indirect_dma=True)
    in_l = eng.lower_ap_dma(in_ap, for_indirect_dma=True)
    assert len(in_l) == 1 and len(out_l) == 1
    off_l = eng.lower_ap_dma(offset_ap)
    assert len(off_l) == 1
    in_l.append(off_l[0])
    coef = 1
    for i in range(1, len(in_ap.shape)):
        coef *= in_ap.shape[i]
    in_l[0].dynamic_ap_info = mybir.DynamicAccessPatternInfo(
        c=0,
        actual_ap=out_ap.ap,
        indirect_dim_max_index=in_ap.shape[0],
        offset_expr=[
            mybir.DynamicAccessPatternOffsetExpr(
                coef=coef,
                aff_expr=mybir.DynamicAccessPatternOffsetExprAffExpr(
                    kind="IndirectArgId", arg_id=1
                ),
            )
        ],
    )
    return eng.add_instruction(
        mybir.InstDMACopy(
            name=eng.bass.get_next_instruction_name(),
            queue=f"qPoolDynamic{queue_num or ''}",
            mode="Copy",
            ins=in_l,
            outs=out_l,
            oob_is_err=oob_is_err,
            cce_op=mybir.AluOpType.bypass,
        )
    )


def _build_nc_tile():
    nc = bacc.Bacc(
        "TRN2",
        target_bir_lowering=False,
        debug=False,
        num_devices=N_CORES,
        num_swdge_queues=N_SWDGE_QUEUES,
    )
    x = nc.dram_tensor("x", [LENGTH * HALF, HALFW], DT, kind="ExternalInput").ap()
    idx = nc.dram_tensor(
        "idx", [128, T * HALF], mybir.dt.int32, kind="ExternalInput"
    ).ap()
    out = nc.dram_tensor(
        "out", [128, T * WIDTH], DT, kind="ExternalOutput"
    ).ap()

    with tile.TileContext(nc) as tc:
        with (
            tc.tile_pool(name="idxp", bufs=1) as idxp,
            tc.tile_pool(name="io", bufs=T) as io,
        ):
            idx_tile = idxp.tile([128, T * HALF], mybir.dt.int32)
            idx_eng = nc.gpsimd if IDX_ON_GPSIMD else nc.scalar
            idx_eng.dma_start(out=idx_tile[:], in_=idx[:, :])
            rings = [nc.sync, nc.scalar]
            n_pieces = T * HALF
            for pi in range(n_pieces):
                t, h = divmod(pi, HALF)
                if h == 0:
                    g = io.tile([128, WIDTH], DT, tag="g", name=f"g{t}")
                _indirect_gather(
                    nc.gpsimd,
                    g[:, h * HALFW : (h + 1) * HALFW],
                    x[:, :],
                    idx_tile[:, pi : pi + 1],
                    queue_num=pi % N_SWDGE_QUEUES,
                )
                col = slice(t * WIDTH + h * HALFW, t * WIDTH + (h + 1) * HALFW)
                gcol = slice(h * HALFW, (h + 1) * HALFW)
                if pi == n_pieces - 1:
                    # final piece: 4-way split over 3 feeds
                    for (lo, hi), eng in zip(
                        [(0, 32), (32, 64), (64, 96), (96, 128)],
                        [nc.sync, nc.scalar, nc.gpsimd, nc.gpsimd],
                    ):
                        eng.dma_start(out=out[lo:hi, col], in_=g[lo:hi, gcol])
                else:
                    a, b = rings[pi % 2], rings[(pi + 1) % 2]
                    a.dma_start(out=out[0:64, col], in_=g[0:64, gcol])
                    b.dma_start(out=out[64:128, col], in_=g[64:128, gcol])
    if STRIP_INIT_BARRIER:
        _strip_init_barrier(nc)
    nc.compile()
    return nc


def _build_nc_dram():
    """Single-pass HBM->HBM gather (no SBUF bounce). Experimental: the
    public API asserts DRAM dest is unsupported; build the instruction
    directly and let correctness testing judge. Each gather targets its
    own offset-0 fully-contiguous output tensor (out{t}[p] = row for
    idx_tile[p, t]) to keep the dst AP maximally simple."""
    nc = bacc.Bacc(
        "TRN2",
        target_bir_lowering=False,
        debug=False,
        num_devices=N_CORES,
        num_swdge_queues=N_SWDGE_QUEUES,
    )
    x = nc.dram_tensor("x", [LENGTH, WIDTH], DT, kind="ExternalInput").ap()
    idx = nc.dram_tensor("idx", [128, T], mybir.dt.int32, kind="ExternalInput").ap()
    outs = [
        nc.dram_tensor(f"out{t}", [128, WIDTH], DT, kind="ExternalOutput").ap()
        for t in range(T)
    ]

    with tile.TileContext(nc) as tc:
        with tc.tile_pool(name="idxp", bufs=1) as idxp:
            idx_tile = idxp.tile([128, T], mybir.dt.int32)
            idx_eng = nc.gpsimd if IDX_ON_GPSIMD else nc.scalar
            idx_eng.dma_start(out=idx_tile[:], in_=idx[:, :])
            for t in range(T):
                _indirect_gather(
                    nc.gpsimd,
                    outs[t][:, :],
                    x[:, :],
                    idx_tile[:, t : t + 1],
                    queue_num=t % N_SWDGE_QUEUES,
                    oob_is_err=False,
                )
    if STRIP_INIT_BARRIER:
        _strip_init_barrier(nc)
    nc.compile()
    return nc


def _build_nc():
    global _nc_cache
    if _nc_cache is None:
        _nc_cache = _build_nc_dram() if MODE == "dram" else _build_nc_tile()
    return _nc_cache


def _shard_inputs(inputs: np.ndarray, idx: np.ndarray):
    # interleave batches: x_il[l] = [x[0,l,:], x[1,l,:], x[2,l,:], x[3,l,:]]
    x_il = np.ascontiguousarray(
        inputs.transpose(1, 0, 2).reshape(LENGTH, WIDTH).astype(NP_DT)
    )
    in_maps = []
    for k in range(N_CORES):
        chunk = idx[k * ROWS_PER_CORE : (k + 1) * ROWS_PER_CORE]
        a = chunk.reshape(128, T).astype(np.int32)
        # half-row indices: columns (2t, 2t+1) = (2*idx, 2*idx+1)
        shard = np.ascontiguousarray(
            np.stack([HALF * a + j for j in range(HALF)], axis=-1).reshape(
                128, T * HALF
            )
        )
        in_maps.append({"x": x_il, "idx": shard})
    return in_maps


def _run(inputs: np.ndarray, idx: np.ndarray, **run_kwargs):
    nc = _build_nc()
    in_maps = _shard_inputs(inputs, idx)
    res = run_bass_kernel_spmd(nc, in_maps, list(range(N_CORES)), **run_kwargs)
    out = np.empty((B, CAP, EMBED), np.float32)
    for k in range(N_CORES):
        if MODE == "dram":
            # out{t}[p] = row p*T + t -> stack to [128, T, B, EMBED]
            arr = np.stack(
                [
                    np.asarray(res.results[k][f"out{t}"]).reshape(128, B, EMBED)
                    for t in range(T)
                ],
                axis=1,
            )
        else:
            arr = np.asarray(res.results[k]["out"]).reshape(128, T, B, EMBED)
        out[:, k * ROWS_PER_CORE : (k + 1) * ROWS_PER_CORE] = (
            arr.transpose(2, 0, 1, 3).reshape(B, ROWS_PER_CORE, EMBED)
        ).astype(np.float32)
    return out, res


def kernel(inputs: np.ndarray, idx: np.ndarray) -> np.ndarray:
    inputs = np.asarray(inputs, dtype=np.float32)
    idx = np.asarray(idx, dtype=np.int32)
    out, _ = _run(inputs, idx)
    return out


# revision 31
# speedup vs baseline: 1.1635x; 1.1635x over previous
"""DropToken gather kernel for Trainium2 (8 NeuronCores).

Computes out[b, c, :] = inputs[b, idx[c], :] (the reference's one-hot
matmul is just a row gather). Memory-bound.

Key layout trick: all 4 batches share idx, so the host interleaves
batches into x_il[l] = concat(x[0,l], x[1,l], x[2,l], x[3,l]) — one 8KB
fp16 row per token. One SWDGE gather descriptor then fetches the row
for all 4 batches at once: 512 descriptors/core instead of 2048, and
8KB packets keep the 16 SDMA engines at full per-packet efficiency.

Precision: correctness gate is rel_err < 2e-2; fp16 rounding costs
~5e-4 while halving every DMA byte. Host casts f32->f16 and back.

Sharding: core k handles output rows [k*512, (k+1)*512) of the cap dim
for all batches. Slot (p, t) of the [128, T=4] layout = row p*T + t.
"""

import numpy as np

import concourse.bass as bass
import concourse.tile as tile
from concourse import bacc, mybir
from concourse.bass_utils import run_bass_kernel_spmd

B = 4
LENGTH = 8192
EMBED = 1024
CAP = 4096
N_CORES = 8
WIDTH = B * EMBED  # interleaved row width (elements)
ROWS_PER_CORE = CAP // N_CORES  # 512 cap rows per core
T = ROWS_PER_CORE // 128  # 4 gathered rows per partition

DT = mybir.dt.float16
NP_DT = np.float16

_nc_cache = None
STRIP_INIT_BARRIER = True
MODE = "tile"  # "dram" = single-pass HBM->HBM gather (crashes: SWDGE ucode
               # computes partition-style dst addresses; DRAM dest unsupported)
N_SWDGE_QUEUES = 2
IDX_ON_GPSIMD = False

# NOTE: the SWDGE queue feed is packet-rate-bound (~30 packets/us), so
# 8KB full-row descriptors are mandatory — splitting rows halves feed
# bandwidth (measured 40.9us vs 35.2us).
HALF = 1
HALFW = WIDTH // HALF

# Gathers stay full-width (SWDGE offset APs at partition offsets crash
# the runtime). Stores: the 16 SDMA engines are saturated mid-kernel
# (~360 GB/s DMA bus), so what matters is the tail — every store piece is
# partition-split across the sync/scalar HWDGE rings, and the final
# half-column also borrows gpsimd's SWDGE queue as a third feed (its
# descriptor-gen work is long done by then).


def _strip_init_barrier(nc):
    """Remove the Bass-init const memsets and all-engine barrier from the
    entry block. This kernel has no cross-engine deps besides DMA
    semaphores (runtime-zeroed at NEFF load), so engine-boot alignment is
    unnecessary; saves ~3us of startup."""
    blk = nc.m.functions[0].blocks[0]
    blk.instructions = [
        ins
        for ins in blk.instructions
        if not isinstance(
            ins, (mybir.InstMemset, mybir.InstDrain, mybir.InstEventSemaphore)
        )
    ]


def _indirect_gather(eng, out_ap, in_ap, offset_ap, queue_num=0, oob_is_err=True):
    """Indirect gather (one offset per partition) pinned to
    qPoolDynamic{queue_num}, allowing any (incl. DRAM) destination AP.
    Mirrors bass's indirect_dma_start gather-arm lowering."""
    out_l = eng.lower_ap_dma(out_ap, for_indirect_dma=True)
    in_l = eng.lower_ap_dma(in_ap, for_indirect_dma=True)
    assert len(in_l) == 1 and len(out_l) == 1
    off_l = eng.lower_ap_dma(offset_ap)
    assert len(off_l) == 1
    in_l.append(off_l[0])
    coef = 1
    for i in range(1, len(in_ap.shape)):
        coef *= in_ap.shape[i]
    in_l[0].dynamic_ap_info = mybir.DynamicAccessPatternInfo(
        c=0,
        actual_ap=out_ap.ap,
        indirect_dim_max_index=in_ap.shape[0],
        offset_expr=[
            mybir.DynamicAccessPatternOffsetExpr(
                coef=coef,
                aff_expr=mybir.DynamicAccessPatternOffsetExprAffExpr(
                    kind="IndirectArgId", arg_id=1
                ),
            )
        ],
    )
    return eng.add_instruction(
        mybir.InstDMACopy(
            name=eng.bass.get_next_instruction_name(),
            queue=f"qPoolDynamic{queue_num or ''}",
            mode="Copy",
            ins=in_l,
            outs=out_l,
            oob_is_err=oob_is_err,
            cce_op=mybir.AluOpType.bypass,
        )
    )


def _build_nc_tile():
    nc = bacc.Bacc(
        "TRN2",
        target_bir_lowering=False,
        debug=False,
        num_devices=N_CORES,
        num_swdge_queues=N_SWDGE_QUEUES,
    )
    x = nc.dram_tensor("x", [LENGTH * HALF, HALFW], DT, kind="ExternalInput").ap()
    idx = nc.dram_tensor(
        "idx", [128, T * HALF], mybir.dt.int32, kind="ExternalInput"
    ).ap()
    out = nc.dram_tensor(
        "out", [128, T * WIDTH], DT, kind="ExternalOutput"
    ).ap()

    with tile.TileContext(nc) as tc:
        with (
            tc.tile_pool(name="idxp", bufs=1) as idxp,
            tc.tile_pool(name="io", bufs=T) as io,
        ):
            idx_tile = idxp.tile([128, T * HALF], mybir.dt.int32)
            idx_eng = nc.gpsimd if IDX_ON_GPSIMD else nc.scalar
            idx_eng.dma_start(out=idx_tile[:], in_=idx[:, :])
            rings = [nc.sync, nc.scalar]
            n_pieces = T * HALF
            for pi in range(n_pieces):
                t, h = divmod(pi, HALF)
                if h == 0:
                    g = io.tile([128, WIDTH], DT, tag="g", name=f"g{t}")
                _indirect_gather(
                    nc.gpsimd,
                    g[:, h * HALFW : (h + 1) * HALFW],
                    x[:, :],
                    idx_tile[:, pi : pi + 1],
                    queue_num=pi % N_SWDGE_QUEUES,
                )
                col = slice(t * WIDTH + h * HALFW, t * WIDTH + (h + 1) * HALFW)
                gcol = slice(h * HALFW, (h + 1) * HALFW)
                if pi == n_pieces - 1:
                    # final piece: 4-way split over 3 feeds
                    for (lo, hi), eng in zip(
                        [(0, 32), (32, 64), (64, 96), (96, 128)],
                        [nc.sync, nc.scalar, nc.gpsimd, nc.gpsimd],
                    ):
                        eng.dma_start(out=out[lo:hi, col], in_=g[lo:hi, gcol])
                else:
                    a, b = rings[pi % 2], rings[(pi + 1) % 2]
                    a.dma_start(out=out[0:64, col], in_=g[0:64, gcol])
                    b.dma_start(out=out[64:128, col], in_=g[64:128, gcol])
    if STRIP_INIT_BARRIER:
        _strip_init_barrier(nc)
    nc.compile()
    return nc


def _build_nc_dram():
    """Single-pass HBM->HBM gather (no SBUF bounce). Experimental: the
    public API asserts DRAM dest is unsupported; build the instruction
    directly and let correctness testing judge. Each gather targets its
    own offset-0 fully-contiguous output tensor (out{t}[p] = row for
    idx_tile[p, t]) to keep the dst AP maximally simple."""
    nc = bacc.Bacc(
        "TRN2",
        target_bir_lowering=False,
        debug=False,
        num_devices=N_CORES,
        num_swdge_queues=N_SWDGE_QUEUES,
    )
    x = nc.dram_tensor("x", [LENGTH, WIDTH], DT, kind="ExternalInput").ap()
    idx = nc.dram_tensor("idx", [128, T], mybir.dt.int32, kind="ExternalInput").ap()
    outs = [
        nc.dram_tensor(f"out{t}", [128, WIDTH], DT, kind="ExternalOutput").ap()
        for t in range(T)
    ]

    with tile.TileContext(nc) as tc:
        with tc.tile_pool(name="idxp", bufs=1) as idxp:
            idx_tile = idxp.tile([128, T], mybir.dt.int32)
            idx_eng = nc.gpsimd if IDX_ON_GPSIMD else nc.scalar
            idx_eng.dma_start(out=idx_tile[:], in_=idx[:, :])
            for t in range(T):
                _indirect_gather(
                    nc.gpsimd,
                    outs[t][:, :],
                    x[:, :],
                    idx_tile[:, t : t + 1],
                    queue_num=t % N_SWDGE_QUEUES,
                    oob_is_err=False,
                )
    if STRIP_INIT_BARRIER:
        _strip_init_barrier(nc)
    nc.compile()
    return nc


def _build_nc():
    global _nc_cache
    if _nc_cache is None:
        _nc_cache = _build_nc_dram() if MODE == "dram" else _build_nc_tile()
    return _nc_cache


def _shard_inputs(inputs: np.ndarray, idx: np.ndarray):
    # interleave batches: x_il[l] = [x[0,l,:], x[1,l,:], x[2,l,:], x[3,l,:]]
    x_il = np.ascontiguousarray(
        inputs.transpose(1, 0, 2).reshape(LENGTH, WIDTH).astype(NP_DT)
    )
    in_maps = []
    for k in range(N_CORES):
        chunk = idx[k * ROWS_PER_CORE : (k + 1) * ROWS_PER_CORE]
        a = chunk.reshape(128, T).astype(np.int32)
        # half-row indices: columns (2t, 2t+1) = (2*idx, 2*idx+1)
        shard = np.ascontiguousarray(
            np.stack([HALF * a + j for j in range(HALF)], axis=-1).reshape(
                128, T * HALF
            )
        )
        in_maps.append({"x": x_il, "idx": shard})
    return in_maps


def _run(inputs: np.ndarray, idx: np.ndarray, **run_kwargs):
    nc = _build_nc()
    in_maps = _shard_inputs(inputs, idx)
    res = run_bass_kernel_spmd(nc, in_maps, list(range(N_CORES)), **run_kwargs)
    out = np.empty((B, CAP, EMBED), np.float32)
    for k in range(N_CORES):
        if MODE == "dram":
            # out{t}[p] = row p*T + t -> stack to [128, T, B, EMBED]
            arr = np.stack(
                [
                    np.asarray(res.results[k][f"out{t}"]).reshape(128, B, EMBED)
                    for t in range(T)
                ],
                axis=1,
            )
        else:
            arr = np.asarray(res.results[k]["out"]).reshape(128, T, B, EMBED)
        out[:, k * ROWS_PER_CORE : (k + 1) * ROWS_PER_CORE] = (
            arr.transpose(2, 0, 1, 3).reshape(B, ROWS_PER_CORE, EMBED)
        ).astype(np.float32)
    return out, res


def kernel(inputs: np.ndarray, idx: np.ndarray) -> np.ndarray:
    inputs = np.asarray(inputs, dtype=np.float32)
    idx = np.asarray(idx, dtype=np.int32)
    out, _ = _run(inputs, idx)
    return out
